# revision 35
# baseline (speedup 1.0000x reference)
"""Trainium2 Bass kernel for a dense transformer block (attention + MLP).

Reference shapes: x [4, 1024, 1024], H=16 heads, DH=64, MLP=4096.

Distribution (8 cores, no collectives): core i handles batch b = i//2,
sequence half h = i%2 (512 query tokens).  Each core receives the full
1024-token context of its batch element with its own 512 tokens permuted
first, recomputes K/V for the whole context (+14% FLOPs, zero comm), and
produces its 512 output rows.  Softmax is order-invariant over keys, so the
permuted context gives exact results.

Per-core pipeline (all matmuls bf16 with f32 PSUM accumulation):
  LN1 (bn_stats) -> DMA-transpose -> V,K,Q projections -> per head:
  S^T = K_h^T.Q_h, exp (scores bounded, no max-subtraction), oT_h = V'_h.A
  (V augmented with a ones column so the softmax denominator is row DH of
  the same accumulation), normalize via a K=1 fp32r broadcast matmul ->
  O-proj + residual -> LN2 -> MLP1 + exact erf-gelu -> MLP2 + residual.

Host-side folds: LN affine into the following weight matrices, attention
scale into W_q/b_q, b_o into the residual input, 0.5 of gelu into W2.
"""

import os
import numpy as np
import ml_dtypes

import concourse.bacc as bacc
import concourse.mybir as mybir
import concourse.tile as tile
from concourse.bass_utils import run_bass_kernel_spmd

B, P, D = 4, 1024, 1024
H, DH = 16, 64
MLP = 4096
SCALE = DH ** -0.5
N_CORES = 8
OWN = P // 2          # query tokens per core
NT = P // 128         # token tiles in context (8)
NQ = OWN // 128       # query token tiles (4)
ND = D // 128         # feature tiles (8)
NM = MLP // 128       # mlp tiles (32)

F32 = mybir.dt.float32
F32R = mybir.dt.float32r
BF16 = mybir.dt.bfloat16
AF = mybir.ActivationFunctionType
ALU = mybir.AluOpType
AX = mybir.AxisListType

_CACHE = {}
_PHASES = os.environ.get("KBENCH_PHASES", "ABEFGH")


def _build():
    nc = bacc.Bacc(None, target_bir_lowering=False, debug=False)

    x = nc.declare_dram_parameter("x", [P, D], BF16, isOutput=False)
    xres = nc.declare_dram_parameter("xres", [OWN, D], F32, isOutput=False)
    wqkv = nc.declare_dram_parameter("wqkv", [D, 3 * D], BF16, isOutput=False)
    bqk = nc.declare_dram_parameter("bqk", [128, 16], F32, isOutput=False)
    bv = nc.declare_dram_parameter("bv", [128, D], F32, isOutput=False)
    wo = nc.declare_dram_parameter("wo", [D, D], BF16, isOutput=False)
    w1 = nc.declare_dram_parameter("w1", [D, MLP], BF16, isOutput=False)
    b1 = nc.declare_dram_parameter("b1", [128, NM], F32, isOutput=False)
    b1s = nc.declare_dram_parameter("b1s", [128, NM], F32, isOutput=False)
    w2 = nc.declare_dram_parameter("w2", [MLP, D], BF16, isOutput=False)
    b2 = nc.declare_dram_parameter("b2", [128, D], F32, isOutput=False)
    out = nc.declare_dram_parameter("out", [OWN, D], F32, isOutput=True)

    with tile.TileContext(nc) as tc:
        with tc.tile_pool(name="persist", bufs=1) as PP:
            # feature-major slabs: slab[p, d, t] = mat[t, d*128+p]
            xnT = PP.tile([128, ND, P], BF16)       # LN1(x) transposed
            qT = PP.tile([128, ND, OWN], BF16)      # queries (pre-scaled)
            kT = PP.tile([128, ND, P], BF16)        # keys
            vA = PP.tile([128, NT, H * (DH + 1)], BF16)  # values + ones col/head
            oT = PP.tile([128, ND, OWN], BF16)      # attn out, feature-major
            x2 = PP.tile([128, NQ, D], F32)         # attn residual state
            x2b = PP.tile([128, NQ, D], F32)        # x2 + b2 (final residual)
            xn2T = PP.tile([128, ND, OWN], BF16)    # LN2(x2) transposed
            bqk_sb = PP.tile([128, 16], F32)
            bv_sb = PP.tile([128, D], F32)
            b1_sb = PP.tile([128, NM], F32)
            b1s_sb = PP.tile([128, NM], F32)
            b2_sb = PP.tile([128, D], F32)
            eps = PP.tile([128, 1], F32)
            nc.vector.memset(eps[:], 1e-5)
            ones64 = PP.tile([1, DH], BF16)
            nc.vector.memset(ones64[:], 1.0)

            nc.sync.dma_start(bqk_sb[:], bqk[:])
            nc.sync.dma_start(bv_sb[:], bv[:])

            def ln_tile(pool, src, dst_slab, col):
                """LayerNorm src [128, D] (f32) -> bf16, DMA-transposed into
                dst_slab[:, :, col*128:(col+1)*128]."""
                st6 = pool.tile([128, 2, 6], F32, tag="st6", name="st6")
                for c in range(2):
                    nc.vector.bn_stats(st6[:, c, :], src[:, c * 512:(c + 1) * 512])
                mv = pool.tile([128, 2], F32, tag="mv", name="mv")
                nc.vector.bn_aggr(mv[:], st6[:, :, :].rearrange("p a b -> p (a b)"))
                std = pool.tile([128, 1], F32, tag="std", name="std")
                nc.scalar.activation(std[:], mv[:, 1:2], AF.Sqrt,
                                     bias=eps[:, 0:1], scale=1.0)
                rstd = pool.tile([128, 1], F32, tag="rstd", name="rstd")
                nc.vector.reciprocal(rstd[:], std[:])
                nmr = pool.tile([128, 1], F32, tag="nmr", name="nmr")
                nc.vector.scalar_tensor_tensor(nmr[:], mv[:, 0:1], -1.0, rstd[:],
                                               op0=ALU.mult, op1=ALU.mult)
                xnb = pool.tile([128, D], BF16, tag="xnb", name="xnb")
                nc.scalar.activation(xnb[:], src, AF.Identity,
                                     bias=nmr[:, 0:1], scale=rstd[:, 0:1])
                nc.sync.dma_start(dst_slab[:, :, col * 128:(col + 1) * 128],
                                  xnb[:], transpose=True)

            # ------- Phase A: LN1 + transpose + V projection (fused) -------
            if "A" in _PHASES:
                with (
                    nc.named_scope("phA_ln1_v"),
                    tc.tile_pool(name="phA", bufs=3) as PA,
                    tc.tile_pool(name="wstA", bufs=1) as WSA,
                    tc.tile_pool(name="psA", bufs=4, space="PSUM") as PSA,
                ):
                    xts = []
                    for tt in range(NT):
                        xt = PA.tile([128, D], BF16, tag="xt", name=f"xt{tt}",
                                     bufs=NT)
                        nc.sync.dma_start(xt[:], x[tt * 128:(tt + 1) * 128, :])
                        xts.append(xt)
                    wv = WSA.tile([128, ND, D], BF16)
                    for kt in range(ND):
                        nc.sync.dma_start(wv[:, kt, :],
                                          wqkv[kt * 128:(kt + 1) * 128,
                                               2 * D:3 * D])
                    vA4 = vA.rearrange("p t (h e) -> p t h e", e=DH + 1)
                    nc.vector.memset(vA4[:, :, :, DH:DH + 1], 1.0)
                    for tt in range(NT):
                        ln_tile(PA, xts[tt][:], xnT, tt)
                        pss = [PSA.tile([128, 512], F32, tag="psA",
                                        name=f"psv{c}") for c in range(2)]
                        for kt in range(ND):
                            for c in range(2):
                                nc.tensor.matmul(
                                    pss[c][:],
                                    xnT[:, kt, tt * 128:(tt + 1) * 128],
                                    wv[:, kt, c * 512:(c + 1) * 512],
                                    start=(kt == 0), stop=(kt == ND - 1))
                        for c in range(2):
                            nc.vector.tensor_add(
                                vA4[:, tt, c * 8:(c + 1) * 8, 0:DH],
                                pss[c][:].rearrange("p (h e) -> p h e", e=DH),
                                bv_sb[:, c * 512:(c + 1) * 512].rearrange(
                                    "p (h e) -> p h e", e=DH))

            # ------- Phase B: K/Q projection + attention (fused per ft) -------
            if "B" in _PHASES:
                with (
                    nc.named_scope("phB_kq_attn"),
                    tc.tile_pool(name="wstage", bufs=2) as WS,
                    tc.tile_pool(name="attn", bufs=2) as AT,
                    tc.tile_pool(name="small", bufs=4) as SM,
                    tc.tile_pool(name="pskq", bufs=2, space="PSUM") as PSB,
                    tc.tile_pool(name="psS", bufs=2, space="PSUM") as PSS,
                    tc.tile_pool(name="psO", bufs=1, space="PSUM") as PSO,
                ):
                    wk = WS.tile([128, ND, D], BF16, tag="wreg", name="wk")
                    for kt in range(ND):
                        nc.sync.dma_start(wk[:, kt, :],
                                          wqkv[kt * 128:(kt + 1) * 128, D:2 * D])
                    wq = WS.tile([128, ND, D], BF16, tag="wreg", name="wq")
                    for kt in range(ND):
                        nc.sync.dma_start(wq[:, kt, :],
                                          wqkv[kt * 128:(kt + 1) * 128, 0:D])
                    for ft in range(ND):
                        # K[ft] (both token chunks) and Q[ft]
                        psk = [PSB.tile([128, 512], F32, tag="pskq",
                                        name=f"psk{c}") for c in range(2)]
                        for kt in range(ND):
                            for c in range(2):
                                nc.tensor.matmul(
                                    psk[c][:],
                                    wk[:, kt, ft * 128:(ft + 1) * 128],
                                    xnT[:, kt, c * 512:(c + 1) * 512],
                                    start=(kt == 0), stop=(kt == ND - 1))
                        for c in range(2):
                            nc.scalar.activation(
                                kT[:, ft, c * 512:(c + 1) * 512], psk[c][:],
                                AF.Identity, bias=bqk_sb[:, 8 + ft:9 + ft],
                                scale=1.0)
                        psq = PSB.tile([128, OWN], F32, tag="pskq", name="psq")
                        for kt in range(ND):
                            nc.tensor.matmul(psq[:],
                                             wq[:, kt, ft * 128:(ft + 1) * 128],
                                             xnT[:, kt, 0:OWN],
                                             start=(kt == 0), stop=(kt == ND - 1))
                        nc.scalar.activation(qT[:, ft, :], psq[:], AF.Identity,
                                             bias=bqk_sb[:, ft:ft + 1], scale=1.0)
                        # attention for the two heads living in this ft tile
                        for hh in range(2):
                            h = 2 * ft + hh
                            off = hh * DH
                            kh = kT[off:off + DH, ft, :]
                            qh = qT[off:off + DH, ft, :]
                            expS = AT.tile([128, NT, OWN], BF16, tag="expS",
                                           name="expS", bufs=3)
                            for kp in range(NT // 2):
                                ps = PSS.tile([128, 2, OWN], F32, tag="psS",
                                              name="psS")
                                for j in range(2):
                                    kt = 2 * kp + j
                                    nc.tensor.matmul(
                                        ps[:, j, :],
                                        kh[:, kt * 128:(kt + 1) * 128],
                                        qh[:, :], start=True, stop=True)
                                nc.scalar.activation(
                                    expS[:, 2 * kp:2 * kp + 2, :].rearrange(
                                        "p a b -> p (a b)"),
                                    ps[:, :, :].rearrange("p a b -> p (a b)"),
                                    AF.Exp)
                            # oT_h[d, q] (+ row DH = softmax denominator)
                            po = PSO.tile([DH + 1, OWN], F32, tag="psO",
                                          name="psO")
                            for kt in range(NT):
                                nc.tensor.matmul(
                                    po[:],
                                    vA[:, kt, h * (DH + 1):(h + 1) * (DH + 1)],
                                    expS[:, kt, :],
                                    start=(kt == 0), stop=(kt == NT - 1))
                            rec = SM.tile([1, OWN], F32, tag="rec", name="rec")
                            nc.vector.reciprocal(rec[:], po[DH:DH + 1, :])
                            recb = SM.tile([1, OWN], BF16, tag="recb",
                                           name="recb")
                            nc.scalar.copy(recb[:], rec[:])
                            pb = PSO.tile([DH, OWN], F32, tag="psBC",
                                          name="psBC", bufs=1)
                            nc.tensor.matmul(pb[:], ones64[:], recb[:],
                                             start=True, stop=True)
                            nb = SM.tile([DH, OWN], F32, tag="nb", name="nb")
                            nc.vector.tensor_copy(nb[:], pb[:])
                            nc.vector.tensor_mul(oT[off:off + DH, ft, :],
                                                 po[0:DH, :], nb[:])

            # ---------------- Phase E: O-proj + residual ----------------
            if "E" in _PHASES:
                with (
                    nc.named_scope("phE_oproj"),
                    tc.tile_pool(name="phE", bufs=1) as PE_,
                    tc.tile_pool(name="psE", bufs=4, space="PSUM") as PSE,
                ):
                    wo_sb = PE_.tile([128, ND, D], BF16)
                    for kt in range(ND):
                        nc.sync.dma_start(wo_sb[:, kt, :],
                                          wo[kt * 128:(kt + 1) * 128, :])
                    xres_sb = PE_.tile([128, NQ, D], F32)
                    for qt in range(NQ):
                        nc.sync.dma_start(xres_sb[:, qt, :],
                                          xres[qt * 128:(qt + 1) * 128, :])
                    for qt in range(NQ):
                        for c in range(2):
                            ps = PSE.tile([128, 512], F32, tag="psE", name="psE")
                            for kt in range(ND):
                                nc.tensor.matmul(
                                    ps[:], oT[:, kt, qt * 128:(qt + 1) * 128],
                                    wo_sb[:, kt, c * 512:(c + 1) * 512],
                                    start=(kt == 0), stop=(kt == ND - 1))
                            nc.vector.tensor_add(
                                x2[:, qt, c * 512:(c + 1) * 512], ps[:],
                                xres_sb[:, qt, c * 512:(c + 1) * 512])

            nc.sync.dma_start(b2_sb[:], b2[:])
            nc.sync.dma_start(b1_sb[:], b1[:])
            nc.sync.dma_start(b1s_sb[:], b1s[:])

            # ---------------- Phase F: LN2 + transpose + x2b ----------------
            if "F" in _PHASES:
                with nc.named_scope("phF_ln2"), \
                        tc.tile_pool(name="phF", bufs=2) as PF:
                    for qt in range(NQ):
                        ln_tile(PF, x2[:, qt, :], xn2T, qt)
                        nc.vector.tensor_add(x2b[:, qt, :], x2[:, qt, :],
                                             b2_sb[:])

            # ---------------- Phase G: MLP1 + erf-gelu ----------------
            if "G" in _PHASES:
                with tc.tile_pool(name="mlpG", bufs=1) as MG:
                    g_sb = MG.tile([128, NM, OWN], BF16)
                    with (
                        nc.named_scope("phG_mlp1"),
                        tc.tile_pool(name="w1s", bufs=3) as W1S,
                        tc.tile_pool(name="psG", bufs=8, space="PSUM") as PSG,
                    ):
                        for gp in range(NM // 8):
                            w1ts = []
                            for kt in range(ND):
                                w1t = W1S.tile([128, 1024], BF16, tag="w1t",
                                               name=f"w1t{kt}", bufs=10)
                                nc.sync.dma_start(
                                    w1t[:], w1[kt * 128:(kt + 1) * 128,
                                               gp * 1024:(gp + 1) * 1024])
                                w1ts.append(w1t)
                            for sub in range(2):
                                gm = 2 * gp + sub
                                pss = [PSG.tile([128, OWN], F32, tag="psG",
                                                name=f"psG{mi}")
                                       for mi in range(4)]
                                for kt in range(ND):
                                    for mi in range(4):
                                        nc.tensor.matmul(
                                            pss[mi][:],
                                            w1ts[kt][:, sub * 512 + mi * 128:
                                                     sub * 512 + (mi + 1) * 128],
                                            xn2T[:, kt, :],
                                            start=(kt == 0), stop=(kt == ND - 1))
                                for mi in range(4):
                                    mt = gm * 4 + mi
                                    # exact gelu via erf: g = z*(1+erf(z/sqrt2));
                                    # the 0.5 is folded into W2 on the host.
                                    et = W1S.tile([128, OWN], BF16, tag="et",
                                                  name="et", bufs=4)
                                    nc.scalar.activation(
                                        et[:], pss[mi][:], AF.Erf,
                                        bias=b1s_sb[:, mt:mt + 1],
                                        scale=0.7071067811865476)
                                    zt = W1S.tile([128, OWN], BF16, tag="zt",
                                                  name="zt", bufs=4)
                                    nc.vector.tensor_scalar_add(
                                        zt[:], pss[mi][:], b1_sb[:, mt:mt + 1])
                                    nc.vector.scalar_tensor_tensor(
                                        g_sb[:, mt, :], et[:], 1.0, zt[:],
                                        op0=ALU.add, op1=ALU.mult)

                    # ---------------- Phase H: MLP2 + final residual --------
                    if "H" in _PHASES:
                        with (
                            nc.named_scope("phH_mlp2"),
                            tc.tile_pool(name="w2s", bufs=3) as W2S,
                            tc.tile_pool(name="psH", bufs=8, space="PSUM") as PSH,
                            tc.tile_pool(name="phH", bufs=2) as PH,
                        ):
                            psh = [PSH.tile([128, 512], F32, tag="psH",
                                            name=f"psH{j}") for j in range(8)]
                            for mt in range(NM):
                                w2t = W2S.tile([128, D], BF16, tag="w2t",
                                               name="w2t")
                                nc.sync.dma_start(
                                    w2t[:], w2[mt * 128:(mt + 1) * 128, :])
                                for qt in range(NQ):
                                    for c in range(2):
                                        nc.tensor.matmul(
                                            psh[qt * 2 + c][:],
                                            g_sb[:, mt, qt * 128:(qt + 1) * 128],
                                            w2t[:, c * 512:(c + 1) * 512],
                                            start=(mt == 0), stop=(mt == NM - 1))
                            for qt in range(NQ):
                                ob = PH.tile([128, D], F32, tag="ob", name="ob")
                                for c in range(2):
                                    nc.vector.tensor_add(
                                        ob[:, c * 512:(c + 1) * 512],
                                        psh[qt * 2 + c][:],
                                        x2b[:, qt, c * 512:(c + 1) * 512])
                                nc.sync.dma_start(out[qt * 128:(qt + 1) * 128, :],
                                                  ob[:])

            if "H" not in _PHASES:
                with tc.tile_pool(name="fb", bufs=1) as FB:
                    fbt = FB.tile([128, D], F32)
                    for qt in range(NQ):
                        nc.sync.dma_start(fbt[:], x[qt * 128:(qt + 1) * 128, :])
                        nc.sync.dma_start(out[qt * 128:(qt + 1) * 128, :], fbt[:])

    nc.compile()
    return nc


def _prep_shared(inputs):
    f = lambda k: np.asarray(inputs[k], dtype=np.float32)
    W_qkv, b_qkv = f("W_qkv"), f("b_qkv")
    ln1_g, ln1_b = f("ln1_g"), f("ln1_b")
    ln2_g, ln2_b = f("ln2_g"), f("ln2_b")
    W1, b1 = f("W1"), f("b1")
    W2, b2 = f("W2"), f("b2")
    W_o, b_o = f("W_o"), f("b_o")

    Wq = ln1_g[:, None] * W_qkv
    bq = b_qkv + ln1_b @ W_qkv
    Wq[:, :D] *= SCALE
    bq = bq.copy()
    bq[:D] *= SCALE

    W1e = ln2_g[:, None] * W1
    b1e = b1 + ln2_b @ W1

    bf = ml_dtypes.bfloat16
    return {
        "wqkv": np.ascontiguousarray(Wq, dtype=bf),
        "bqk": np.ascontiguousarray(bq[:2 * D].reshape(16, 128).T,
                                    dtype=np.float32),
        "bv": np.ascontiguousarray(
            np.broadcast_to(bq[2 * D:], (128, D)), dtype=np.float32),
        "wo": np.ascontiguousarray(W_o, dtype=bf),
        "w1": np.ascontiguousarray(W1e, dtype=bf),
        "b1": np.ascontiguousarray(b1e.reshape(NM, 128).T, dtype=np.float32),
        "b1s": np.ascontiguousarray(
            (b1e / np.sqrt(2.0)).reshape(NM, 128).T, dtype=np.float32),
        "w2": np.ascontiguousarray(0.5 * W2, dtype=bf),
        "b2": np.ascontiguousarray(np.broadcast_to(b2, (128, D)),
                                   dtype=np.float32),
    }, b_o


def make_in_maps(inputs):
    shared, b_o = _prep_shared(inputs)
    x = np.asarray(inputs["x"], dtype=np.float32)
    in_maps = []
    for i in range(N_CORES):
        b, h = i // 2, i % 2
        own = x[b, h * OWN:(h + 1) * OWN]
        oth = x[b, (1 - h) * OWN:(2 - h) * OWN]
        m = dict(shared)
        m["x"] = np.ascontiguousarray(
            np.concatenate([own, oth], axis=0), dtype=ml_dtypes.bfloat16)
        m["xres"] = np.ascontiguousarray(own + b_o)
        in_maps.append(m)
    return in_maps


def kernel(**inputs):
    if "nc" not in _CACHE:
        _CACHE["nc"] = _build()
    nc = _CACHE["nc"]
    in_maps = make_in_maps(inputs)
    res = run_bass_kernel_spmd(nc, in_maps, core_ids=list(range(N_CORES)))
    out = np.empty((B, P, D), dtype=np.float32)
    for i in range(N_CORES):
        b, h = i // 2, i % 2
        out[b, h * OWN:(h + 1) * OWN] = res.results[i]["out"]
    return out


# revision 44
# speedup vs baseline: 1.0312x; 1.0312x over previous
"""Trainium2 Bass kernel for a dense transformer block (attention + MLP).

Reference shapes: x [4, 1024, 1024], H=16 heads, DH=64, MLP=4096.

Distribution (8 cores, no collectives): core i handles batch b = i//2,
sequence half h = i%2 (512 query tokens).  Each core receives the full
1024-token context of its batch element with its own 512 tokens permuted
first, recomputes K/V for the whole context (+14% FLOPs, zero comm), and
produces its 512 output rows.  Softmax is order-invariant over keys, so the
permuted context gives exact results.

Per-core pipeline (all matmuls bf16 with f32 PSUM accumulation):
  LN1 (bn_stats) -> DMA-transpose -> V,K,Q projections -> per head:
  S^T = K_h^T.Q_h, exp (scores bounded, no max-subtraction), oT_h = V'_h.A
  (V augmented with a ones column so the softmax denominator is row DH of
  the same accumulation), normalize via a K=1 fp32r broadcast matmul ->
  O-proj + residual -> LN2 -> MLP1 + exact erf-gelu -> MLP2 + residual.

Host-side folds: LN affine into the following weight matrices, attention
scale into W_q/b_q, b_o into the residual input, 0.5 of gelu into W2.
"""

import os
import numpy as np
import ml_dtypes

import concourse.bacc as bacc
import concourse.mybir as mybir
import concourse.tile as tile
from concourse.bass_utils import run_bass_kernel_spmd

B, P, D = 4, 1024, 1024
H, DH = 16, 64
MLP = 4096
SCALE = DH ** -0.5
N_CORES = 8
OWN = P // 2          # query tokens per core
NT = P // 128         # token tiles in context (8)
NQ = OWN // 128       # query token tiles (4)
ND = D // 128         # feature tiles (8)
NM = MLP // 128       # mlp tiles (32)

F32 = mybir.dt.float32
F32R = mybir.dt.float32r
BF16 = mybir.dt.bfloat16
AF = mybir.ActivationFunctionType
ALU = mybir.AluOpType
AX = mybir.AxisListType

_CACHE = {}
_PHASES = os.environ.get("KBENCH_PHASES", "ABEFGH")


def _build():
    nc = bacc.Bacc(None, target_bir_lowering=False, debug=False)

    x = nc.declare_dram_parameter("x", [P, D], BF16, isOutput=False)
    xres = nc.declare_dram_parameter("xres", [OWN, D], F32, isOutput=False)
    wqkv = nc.declare_dram_parameter("wqkv", [D, 3 * D], BF16, isOutput=False)
    bqk = nc.declare_dram_parameter("bqk", [128, 16], F32, isOutput=False)
    bv = nc.declare_dram_parameter("bv", [128, D], F32, isOutput=False)
    wo = nc.declare_dram_parameter("wo", [D, D], BF16, isOutput=False)
    w1 = nc.declare_dram_parameter("w1", [D, MLP], BF16, isOutput=False)
    b1 = nc.declare_dram_parameter("b1", [128, NM], F32, isOutput=False)
    b1s = nc.declare_dram_parameter("b1s", [128, NM], F32, isOutput=False)
    w2 = nc.declare_dram_parameter("w2", [MLP, D], BF16, isOutput=False)
    b2 = nc.declare_dram_parameter("b2", [128, D], F32, isOutput=False)
    out = nc.declare_dram_parameter("out", [OWN, D], F32, isOutput=True)

    with tile.TileContext(nc) as tc:
        with tc.tile_pool(name="persist", bufs=1) as PP:
            # feature-major slabs: slab[p, d, t] = mat[t, d*128+p]
            xnT = PP.tile([128, ND, P], BF16)       # LN1(x) transposed
            qT = PP.tile([128, ND, OWN], BF16)      # queries (pre-scaled)
            kT = PP.tile([128, ND, P], BF16)        # keys
            vA = PP.tile([128, NT, H * (DH + 1)], BF16)  # values + ones col/head
            oT = PP.tile([128, ND, OWN], BF16)      # attn out, feature-major
            x2 = PP.tile([128, NQ, D], F32)         # attn residual state
            x2b = PP.tile([128, NQ, D], F32)        # x2 + b2 (final residual)
            xn2T = PP.tile([128, ND, OWN], BF16)    # LN2(x2) transposed
            bqk_sb = PP.tile([128, 16], F32)
            bv_sb = PP.tile([128, D], F32)
            b1_sb = PP.tile([128, NM], F32)
            b1s_sb = PP.tile([128, NM], F32)
            b2_sb = PP.tile([128, D], F32)
            eps = PP.tile([128, 1], F32)
            nc.vector.memset(eps[:], 1e-5)
            warm = PP.tile([128, 1], F32)
            nc.scalar.sqrt(warm[:], eps[:])
            ones64 = PP.tile([1, DH], BF16)
            nc.vector.memset(ones64[:], 1.0)


            def ln_tile(pool, src, dst_slab, col):
                """LayerNorm src [128, D] (f32) -> bf16, DMA-transposed into
                dst_slab[:, :, col*128:(col+1)*128]."""
                st6 = pool.tile([128, 2, 6], F32, tag="st6", name="st6")
                for c in range(2):
                    nc.vector.bn_stats(st6[:, c, :], src[:, c * 512:(c + 1) * 512])
                mv = pool.tile([128, 2], F32, tag="mv", name="mv")
                nc.vector.bn_aggr(mv[:], st6[:, :, :].rearrange("p a b -> p (a b)"))
                std = pool.tile([128, 1], F32, tag="std", name="std")
                nc.scalar.activation(std[:], mv[:, 1:2], AF.Sqrt,
                                     bias=eps[:, 0:1], scale=1.0)
                rstd = pool.tile([128, 1], F32, tag="rstd", name="rstd")
                nc.vector.reciprocal(rstd[:], std[:])
                nmr = pool.tile([128, 1], F32, tag="nmr", name="nmr")
                nc.vector.scalar_tensor_tensor(nmr[:], mv[:, 0:1], -1.0, rstd[:],
                                               op0=ALU.mult, op1=ALU.mult)
                xnb = pool.tile([128, D], BF16, tag="xnb", name="xnb")
                nc.scalar.activation(xnb[:], src, AF.Identity,
                                     bias=nmr[:, 0:1], scale=rstd[:, 0:1])
                nc.sync.dma_start(dst_slab[:, :, col * 128:(col + 1) * 128],
                                  xnb[:], transpose=True)

            # ------- Phase A: LN1 + transpose + V projection (fused) -------
            if "A" in _PHASES:
                with (
                    nc.named_scope("phA_ln1_v"),
                    tc.tile_pool(name="phA", bufs=3) as PA,
                    tc.tile_pool(name="wstA", bufs=1) as WSA,
                    tc.tile_pool(name="psA", bufs=4, space="PSUM") as PSA,
                ):
                    xts = []
                    for tt in range(NT):
                        xt = PA.tile([128, D], BF16, tag="xt", name=f"xt{tt}",
                                     bufs=NT)
                        if tt < 2:
                            nc.sync.dma_start(xt[:],
                                              x[tt * 128:(tt + 1) * 128, :])
                        xts.append(xt)
                    nc.sync.dma_start(bv_sb[:], bv[:])
                    wv = WSA.tile([128, ND, D], BF16)
                    for kt in range(ND):
                        nc.sync.dma_start(wv[:, kt, :],
                                          wqkv[kt * 128:(kt + 1) * 128,
                                               2 * D:3 * D])
                    for tt in range(2, NT):
                        nc.sync.dma_start(xts[tt][:],
                                          x[tt * 128:(tt + 1) * 128, :])
                    nc.sync.dma_start(bqk_sb[:], bqk[:])
                    vA4 = vA.rearrange("p t (h e) -> p t h e", e=DH + 1)
                    nc.vector.memset(vA4[:, :, :, DH:DH + 1], 1.0)
                    for tt in range(NT):
                        ln_tile(PA, xts[tt][:], xnT, tt)
                        pss = [PSA.tile([128, 512], F32, tag="psA",
                                        name=f"psv{c}") for c in range(2)]
                        for kt in range(ND):
                            for c in range(2):
                                nc.tensor.matmul(
                                    pss[c][:],
                                    xnT[:, kt, tt * 128:(tt + 1) * 128],
                                    wv[:, kt, c * 512:(c + 1) * 512],
                                    start=(kt == 0), stop=(kt == ND - 1))
                        for c in range(2):
                            nc.vector.tensor_add(
                                vA4[:, tt, c * 8:(c + 1) * 8, 0:DH],
                                pss[c][:].rearrange("p (h e) -> p h e", e=DH),
                                bv_sb[:, c * 512:(c + 1) * 512].rearrange(
                                    "p (h e) -> p h e", e=DH))

            # ------- Phase B: K/Q projection + attention (fused per ft) -------
            if "B" in _PHASES:
                with (
                    nc.named_scope("phB_kq_attn"),
                    tc.tile_pool(name="wstage", bufs=2) as WS,
                    tc.tile_pool(name="attn", bufs=2) as AT,
                    tc.tile_pool(name="small", bufs=4) as SM,
                    tc.tile_pool(name="pskq", bufs=2, space="PSUM") as PSB,
                    tc.tile_pool(name="psS", bufs=2, space="PSUM") as PSS,
                    tc.tile_pool(name="psO", bufs=1, space="PSUM") as PSO,
                ):
                    wk = WS.tile([128, ND, D], BF16, tag="wreg", name="wk")
                    for kt in range(ND):
                        nc.sync.dma_start(wk[:, kt, :],
                                          wqkv[kt * 128:(kt + 1) * 128, D:2 * D])
                    wq = WS.tile([128, ND, D], BF16, tag="wreg", name="wq")
                    for kt in range(ND):
                        nc.sync.dma_start(wq[:, kt, :],
                                          wqkv[kt * 128:(kt + 1) * 128, 0:D])
                    for ft in range(ND):
                        # K[ft] (both token chunks) and Q[ft]
                        psk = [PSB.tile([128, 512], F32, tag="pskq",
                                        name=f"psk{c}") for c in range(2)]
                        for kt in range(ND):
                            for c in range(2):
                                nc.tensor.matmul(
                                    psk[c][:],
                                    wk[:, kt, ft * 128:(ft + 1) * 128],
                                    xnT[:, kt, c * 512:(c + 1) * 512],
                                    start=(kt == 0), stop=(kt == ND - 1))
                        for c in range(2):
                            nc.vector.tensor_scalar_add(
                                kT[:, ft, c * 512:(c + 1) * 512], psk[c][:],
                                bqk_sb[:, 8 + ft:9 + ft])
                        psq = PSB.tile([128, OWN], F32, tag="pskq", name="psq")
                        for kt in range(ND):
                            nc.tensor.matmul(psq[:],
                                             wq[:, kt, ft * 128:(ft + 1) * 128],
                                             xnT[:, kt, 0:OWN],
                                             start=(kt == 0), stop=(kt == ND - 1))
                        nc.vector.tensor_scalar_add(qT[:, ft, :], psq[:],
                                                    bqk_sb[:, ft:ft + 1])
                        # attention for the two heads living in this ft tile
                        for hh in range(2):
                            h = 2 * ft + hh
                            off = hh * DH
                            kh = kT[off:off + DH, ft, :]
                            qh = qT[off:off + DH, ft, :]
                            expS = AT.tile([128, NT, OWN], BF16, tag="expS",
                                           name="expS", bufs=3)
                            for kp in range(NT // 2):
                                ps = PSS.tile([128, 2, OWN], F32, tag="psS",
                                              name="psS")
                                for j in range(2):
                                    kt = 2 * kp + j
                                    nc.tensor.matmul(
                                        ps[:, j, :],
                                        kh[:, kt * 128:(kt + 1) * 128],
                                        qh[:, :], start=True, stop=True)
                                nc.scalar.activation(
                                    expS[:, 2 * kp:2 * kp + 2, :].rearrange(
                                        "p a b -> p (a b)"),
                                    ps[:, :, :].rearrange("p a b -> p (a b)"),
                                    AF.Exp)
                            # oT_h[d, q] (+ row DH = softmax denominator)
                            po = PSO.tile([DH + 1, OWN], F32, tag="psO",
                                          name="psO")
                            for kt in range(NT):
                                nc.tensor.matmul(
                                    po[:],
                                    vA[:, kt, h * (DH + 1):(h + 1) * (DH + 1)],
                                    expS[:, kt, :],
                                    start=(kt == 0), stop=(kt == NT - 1))
                            rec = SM.tile([1, OWN], F32, tag="rec", name="rec")
                            nc.vector.reciprocal(rec[:], po[DH:DH + 1, :])
                            recb = SM.tile([1, OWN], BF16, tag="recb",
                                           name="recb")
                            nc.vector.tensor_copy(recb[:], rec[:])
                            pb = PSO.tile([DH, OWN], F32, tag="psBC",
                                          name="psBC", bufs=1)
                            nc.tensor.matmul(pb[:], ones64[:], recb[:],
                                             start=True, stop=True)
                            nb = SM.tile([DH, OWN], F32, tag="nb", name="nb")
                            nc.vector.tensor_copy(nb[:], pb[:])
                            nc.vector.tensor_mul(oT[off:off + DH, ft, :],
                                                 po[0:DH, :], nb[:])

            # ---------------- Phase E: O-proj + residual ----------------
            if "E" in _PHASES:
                with (
                    nc.named_scope("phE_oproj"),
                    tc.tile_pool(name="phE", bufs=1) as PE_,
                    tc.tile_pool(name="psE", bufs=4, space="PSUM") as PSE,
                ):
                    wo_sb = PE_.tile([128, ND, D], BF16)
                    for kt in range(ND):
                        nc.sync.dma_start(wo_sb[:, kt, :],
                                          wo[kt * 128:(kt + 1) * 128, :])
                    xres_sb = PE_.tile([128, NQ, D], F32)
                    for qt in range(NQ):
                        nc.sync.dma_start(xres_sb[:, qt, :],
                                          xres[qt * 128:(qt + 1) * 128, :])
                    for qt in range(NQ):
                        for c in range(2):
                            ps = PSE.tile([128, 512], F32, tag="psE", name="psE")
                            for kt in range(ND):
                                nc.tensor.matmul(
                                    ps[:], oT[:, kt, qt * 128:(qt + 1) * 128],
                                    wo_sb[:, kt, c * 512:(c + 1) * 512],
                                    start=(kt == 0), stop=(kt == ND - 1))
                            nc.vector.tensor_add(
                                x2[:, qt, c * 512:(c + 1) * 512], ps[:],
                                xres_sb[:, qt, c * 512:(c + 1) * 512])

            nc.sync.dma_start(b2_sb[:], b2[:])
            nc.sync.dma_start(b1_sb[:], b1[:])
            nc.sync.dma_start(b1s_sb[:], b1s[:])

            # ---------------- Phase F: LN2 + transpose + x2b ----------------
            if "F" in _PHASES:
                with nc.named_scope("phF_ln2"), \
                        tc.tile_pool(name="phF", bufs=4) as PF:
                    for qt in range(NQ):
                        ln_tile(PF, x2[:, qt, :], xn2T, qt)
                        nc.vector.tensor_add(x2b[:, qt, :], x2[:, qt, :],
                                             b2_sb[:])

            # ---------------- Phase G: MLP1 + erf-gelu ----------------
            if "G" in _PHASES:
                with tc.tile_pool(name="mlpG", bufs=1) as MG:
                    g_sb = MG.tile([128, NM, OWN], BF16)
                    with (
                        nc.named_scope("phG_mlp1"),
                        tc.tile_pool(name="w1s", bufs=3) as W1S,
                        tc.tile_pool(name="psG", bufs=8, space="PSUM") as PSG,
                    ):
                        for gp in range(NM // 8):
                            w1ts = []
                            for kt in range(ND):
                                w1t = W1S.tile([128, 1024], BF16, tag="w1t",
                                               name=f"w1t{kt}", bufs=10)
                                nc.sync.dma_start(
                                    w1t[:], w1[kt * 128:(kt + 1) * 128,
                                               gp * 1024:(gp + 1) * 1024])
                                w1ts.append(w1t)
                            for sub in range(2):
                                gm = 2 * gp + sub
                                pss = [PSG.tile([128, OWN], F32, tag="psG",
                                                name=f"psG{mi}")
                                       for mi in range(4)]
                                for kt in range(ND):
                                    for mi in range(4):
                                        nc.tensor.matmul(
                                            pss[mi][:],
                                            w1ts[kt][:, sub * 512 + mi * 128:
                                                     sub * 512 + (mi + 1) * 128],
                                            xn2T[:, kt, :],
                                            start=(kt == 0), stop=(kt == ND - 1))
                                for mi in range(4):
                                    mt = gm * 4 + mi
                                    # exact gelu via erf: g = z*(1+erf(z/sqrt2));
                                    # the 0.5 is folded into W2 on the host.
                                    et = W1S.tile([128, OWN], BF16, tag="et",
                                                  name="et", bufs=4)
                                    nc.scalar.activation(
                                        et[:], pss[mi][:], AF.Erf,
                                        bias=b1s_sb[:, mt:mt + 1],
                                        scale=0.7071067811865476)
                                    zt = W1S.tile([128, OWN], BF16, tag="zt",
                                                  name="zt", bufs=4)
                                    nc.vector.tensor_scalar_add(
                                        zt[:], pss[mi][:], b1_sb[:, mt:mt + 1])
                                    nc.vector.scalar_tensor_tensor(
                                        g_sb[:, mt, :], et[:], 1.0, zt[:],
                                        op0=ALU.add, op1=ALU.mult)

                    # ---------------- Phase H: MLP2 + final residual --------
                    if "H" in _PHASES:
                        with (
                            nc.named_scope("phH_mlp2"),
                            tc.tile_pool(name="w2s", bufs=3) as W2S,
                            tc.tile_pool(name="psH", bufs=8, space="PSUM") as PSH,
                            tc.tile_pool(name="phH", bufs=2) as PH,
                        ):
                            psh = [PSH.tile([128, 512], F32, tag="psH",
                                            name=f"psH{j}") for j in range(8)]
                            for mt in range(NM):
                                w2t = W2S.tile([128, D], BF16, tag="w2t",
                                               name="w2t")
                                nc.sync.dma_start(
                                    w2t[:], w2[mt * 128:(mt + 1) * 128, :])
                                for qt in range(NQ):
                                    for c in range(2):
                                        nc.tensor.matmul(
                                            psh[qt * 2 + c][:],
                                            g_sb[:, mt, qt * 128:(qt + 1) * 128],
                                            w2t[:, c * 512:(c + 1) * 512],
                                            start=(mt == 0), stop=(mt == NM - 1))
                            for qt in range(NQ):
                                ob = PH.tile([128, D], F32, tag="ob", name="ob")
                                for c in range(2):
                                    nc.vector.tensor_add(
                                        ob[:, c * 512:(c + 1) * 512],
                                        psh[qt * 2 + c][:],
                                        x2b[:, qt, c * 512:(c + 1) * 512])
                                nc.sync.dma_start(out[qt * 128:(qt + 1) * 128, :],
                                                  ob[:])

            if "H" not in _PHASES:
                with tc.tile_pool(name="fb", bufs=1) as FB:
                    fbt = FB.tile([128, D], F32)
                    for qt in range(NQ):
                        nc.sync.dma_start(fbt[:], x[qt * 128:(qt + 1) * 128, :])
                        nc.sync.dma_start(out[qt * 128:(qt + 1) * 128, :], fbt[:])

    nc.compile()
    return nc


def _prep_shared(inputs):
    f = lambda k: np.asarray(inputs[k], dtype=np.float32)
    W_qkv, b_qkv = f("W_qkv"), f("b_qkv")
    ln1_g, ln1_b = f("ln1_g"), f("ln1_b")
    ln2_g, ln2_b = f("ln2_g"), f("ln2_b")
    W1, b1 = f("W1"), f("b1")
    W2, b2 = f("W2"), f("b2")
    W_o, b_o = f("W_o"), f("b_o")

    Wq = ln1_g[:, None] * W_qkv
    bq = b_qkv + ln1_b @ W_qkv
    Wq[:, :D] *= SCALE
    bq = bq.copy()
    bq[:D] *= SCALE

    W1e = ln2_g[:, None] * W1
    b1e = b1 + ln2_b @ W1

    bf = ml_dtypes.bfloat16
    return {
        "wqkv": np.ascontiguousarray(Wq, dtype=bf),
        "bqk": np.ascontiguousarray(bq[:2 * D].reshape(16, 128).T,
                                    dtype=np.float32),
        "bv": np.ascontiguousarray(
            np.broadcast_to(bq[2 * D:], (128, D)), dtype=np.float32),
        "wo": np.ascontiguousarray(W_o, dtype=bf),
        "w1": np.ascontiguousarray(W1e, dtype=bf),
        "b1": np.ascontiguousarray(b1e.reshape(NM, 128).T, dtype=np.float32),
        "b1s": np.ascontiguousarray(
            (b1e / np.sqrt(2.0)).reshape(NM, 128).T, dtype=np.float32),
        "w2": np.ascontiguousarray(0.5 * W2, dtype=bf),
        "b2": np.ascontiguousarray(np.broadcast_to(b2, (128, D)),
                                   dtype=np.float32),
    }, b_o


def make_in_maps(inputs):
    shared, b_o = _prep_shared(inputs)
    x = np.asarray(inputs["x"], dtype=np.float32)
    in_maps = []
    for i in range(N_CORES):
        b, h = i // 2, i % 2
        own = x[b, h * OWN:(h + 1) * OWN]
        oth = x[b, (1 - h) * OWN:(2 - h) * OWN]
        m = dict(shared)
        m["x"] = np.ascontiguousarray(
            np.concatenate([own, oth], axis=0), dtype=ml_dtypes.bfloat16)
        m["xres"] = np.ascontiguousarray(own + b_o)
        in_maps.append(m)
    return in_maps


def kernel(**inputs):
    if "nc" not in _CACHE:
        _CACHE["nc"] = _build()
    nc = _CACHE["nc"]
    in_maps = make_in_maps(inputs)
    res = run_bass_kernel_spmd(nc, in_maps, core_ids=list(range(N_CORES)))
    out = np.empty((B, P, D), dtype=np.float32)
    for i in range(N_CORES):
        b, h = i // 2, i % 2
        out[b, h * OWN:(h + 1) * OWN] = res.results[i]["out"]
    return out


# revision 53
# speedup vs baseline: 1.0374x; 1.0060x over previous
"""Trainium2 Bass kernel for a dense transformer block (attention + MLP).

Reference shapes: x [4, 1024, 1024], H=16 heads, DH=64, MLP=4096.

Distribution (8 cores, no collectives): core i handles batch b = i//2,
sequence half h = i%2 (512 query tokens).  Each core receives the full
1024-token context of its batch element with its own 512 tokens permuted
first, recomputes K/V for the whole context (+14% FLOPs, zero comm), and
produces its 512 output rows.  Softmax is order-invariant over keys, so the
permuted context gives exact results.

Per-core pipeline (all matmuls bf16 with f32 PSUM accumulation):
  LN1 (bn_stats) -> DMA-transpose -> V,K,Q projections -> per head:
  S^T = K_h^T.Q_h, exp (scores bounded, no max-subtraction), oT_h = V'_h.A
  (V augmented with a ones column so the softmax denominator is row DH of
  the same accumulation), normalize via a K=1 fp32r broadcast matmul ->
  O-proj + residual -> LN2 -> MLP1 + exact erf-gelu -> MLP2 + residual.

Host-side folds: LN affine into the following weight matrices, attention
scale into W_q/b_q, b_o into the residual input, 0.5 of gelu into W2.
"""

import os
import numpy as np
import ml_dtypes

import concourse.bacc as bacc
import concourse.mybir as mybir
import concourse.tile as tile
from concourse.bass_utils import run_bass_kernel_spmd

B, P, D = 4, 1024, 1024
H, DH = 16, 64
MLP = 4096
SCALE = DH ** -0.5
N_CORES = 8
OWN = P // 2          # query tokens per core
NT = P // 128         # token tiles in context (8)
NQ = OWN // 128       # query token tiles (4)
ND = D // 128         # feature tiles (8)
NM = MLP // 128       # mlp tiles (32)

F32 = mybir.dt.float32
F32R = mybir.dt.float32r
BF16 = mybir.dt.bfloat16
AF = mybir.ActivationFunctionType
ALU = mybir.AluOpType
AX = mybir.AxisListType

_CACHE = {}
_PHASES = os.environ.get("KBENCH_PHASES", "ABEFGH")


def _build():
    nc = bacc.Bacc(None, target_bir_lowering=False, debug=False)

    x = nc.declare_dram_parameter("x", [P, D], BF16, isOutput=False)
    xres = nc.declare_dram_parameter("xres", [OWN, D], F32, isOutput=False)
    wqkv = nc.declare_dram_parameter("wqkv", [D, 3 * D], BF16, isOutput=False)
    bqk = nc.declare_dram_parameter("bqk", [128, 16], F32, isOutput=False)
    bv = nc.declare_dram_parameter("bv", [128, D], F32, isOutput=False)
    wo = nc.declare_dram_parameter("wo", [D, D], BF16, isOutput=False)
    w1 = nc.declare_dram_parameter("w1", [D, MLP], BF16, isOutput=False)
    b1 = nc.declare_dram_parameter("b1", [128, NM], F32, isOutput=False)
    b1s = nc.declare_dram_parameter("b1s", [128, NM], F32, isOutput=False)
    w2 = nc.declare_dram_parameter("w2", [MLP, D], BF16, isOutput=False)
    b2 = nc.declare_dram_parameter("b2", [128, D], F32, isOutput=False)
    out = nc.declare_dram_parameter("out", [OWN, D], F32, isOutput=True)

    with tile.TileContext(nc) as tc:
        with tc.tile_pool(name="persist", bufs=1) as PP:
            # feature-major slabs: slab[p, d, t] = mat[t, d*128+p]
            xnT = PP.tile([128, ND, P], BF16)       # LN1(x) transposed
            qT = PP.tile([128, ND, OWN], BF16)      # queries (pre-scaled)
            kT = PP.tile([128, ND, P], BF16)        # keys
            vA = PP.tile([128, NT, H * (DH + 1)], BF16)  # values + ones col/head
            oT = PP.tile([128, ND, OWN], BF16)      # attn out, feature-major
            x2 = PP.tile([128, NQ, D], F32)         # attn residual state
            x2b = PP.tile([128, NQ, D], F32)        # x2 + b2 (final residual)
            xn2T = PP.tile([128, ND, OWN], BF16)    # LN2(x2) transposed
            bqk_sb = PP.tile([128, 16], F32)
            bv_sb = PP.tile([128, D], F32)
            b1_sb = PP.tile([128, NM], F32)
            b1s_sb = PP.tile([128, NM], F32)
            b2_sb = PP.tile([128, D], F32)
            eps = PP.tile([128, 1], F32)
            nc.vector.memset(eps[:], 1e-5)
            warm = PP.tile([128, 1], F32)
            nc.scalar.sqrt(warm[:], eps[:])
            ones64 = PP.tile([1, DH], BF16)
            nc.vector.memset(ones64[:], 1.0)


            def ln_tile(pool, src, dst_slab, col):
                """LayerNorm src [128, D] (f32) -> bf16, DMA-transposed into
                dst_slab[:, :, col*128:(col+1)*128]."""
                st6 = pool.tile([128, 2, 6], F32, tag="st6", name="st6")
                for c in range(2):
                    nc.vector.bn_stats(st6[:, c, :], src[:, c * 512:(c + 1) * 512])
                mv = pool.tile([128, 2], F32, tag="mv", name="mv")
                nc.vector.bn_aggr(mv[:], st6[:, :, :].rearrange("p a b -> p (a b)"))
                std = pool.tile([128, 1], F32, tag="std", name="std")
                nc.scalar.activation(std[:], mv[:, 1:2], AF.Sqrt,
                                     bias=eps[:, 0:1], scale=1.0)
                rstd = pool.tile([128, 1], F32, tag="rstd", name="rstd")
                nc.vector.reciprocal(rstd[:], std[:])
                nmr = pool.tile([128, 1], F32, tag="nmr", name="nmr")
                nc.vector.scalar_tensor_tensor(nmr[:], mv[:, 0:1], -1.0, rstd[:],
                                               op0=ALU.mult, op1=ALU.mult)
                xnb = pool.tile([128, D], BF16, tag="xnb", name="xnb")
                nc.scalar.activation(xnb[:], src, AF.Identity,
                                     bias=nmr[:, 0:1], scale=rstd[:, 0:1])
                nc.sync.dma_start(dst_slab[:, :, col * 128:(col + 1) * 128],
                                  xnb[:], transpose=True)

            # ------- Phase A: LN1 + transpose + V projection (fused) -------
            if "A" in _PHASES:
                with (
                    nc.named_scope("phA_ln1_v"),
                    tc.tile_pool(name="phA", bufs=3) as PA,
                    tc.tile_pool(name="wstA", bufs=1) as WSA,
                    tc.tile_pool(name="psA", bufs=4, space="PSUM") as PSA,
                ):
                    xts = []
                    for tt in range(NT):
                        xt = PA.tile([128, D], BF16, tag="xt", name=f"xt{tt}",
                                     bufs=NT)
                        if tt < 2:
                            nc.gpsimd.dma_start(xt[:],
                                                x[tt * 128:(tt + 1) * 128, :])
                        xts.append(xt)
                    nc.gpsimd.dma_start(bv_sb[:], bv[:])
                    wv = WSA.tile([128, ND, D], BF16)
                    for kt in range(ND):
                        nc.sync.dma_start(wv[:, kt, :],
                                          wqkv[kt * 128:(kt + 1) * 128,
                                               2 * D:3 * D])
                    for tt in range(2, NT):
                        nc.gpsimd.dma_start(xts[tt][:],
                                            x[tt * 128:(tt + 1) * 128, :])
                    nc.gpsimd.dma_start(bqk_sb[:], bqk[:])
                    vA4 = vA.rearrange("p t (h e) -> p t h e", e=DH + 1)
                    nc.vector.memset(vA4[:, :, :, DH:DH + 1], 1.0)
                    for tt in range(NT):
                        ln_tile(PA, xts[tt][:], xnT, tt)
                        pss = [PSA.tile([128, 512], F32, tag="psA",
                                        name=f"psv{c}") for c in range(2)]
                        for kt in range(ND):
                            for c in range(2):
                                nc.tensor.matmul(
                                    pss[c][:],
                                    xnT[:, kt, tt * 128:(tt + 1) * 128],
                                    wv[:, kt, c * 512:(c + 1) * 512],
                                    start=(kt == 0), stop=(kt == ND - 1))
                        for c in range(2):
                            nc.vector.tensor_add(
                                vA4[:, tt, c * 8:(c + 1) * 8, 0:DH],
                                pss[c][:].rearrange("p (h e) -> p h e", e=DH),
                                bv_sb[:, c * 512:(c + 1) * 512].rearrange(
                                    "p (h e) -> p h e", e=DH))

            # ------- Phase B: K/Q projection + attention (fused per ft) -------
            if "B" in _PHASES:
                with (
                    nc.named_scope("phB_kq_attn"),
                    tc.tile_pool(name="wstage", bufs=2) as WS,
                    tc.tile_pool(name="attn", bufs=2) as AT,
                    tc.tile_pool(name="small", bufs=4) as SM,
                    tc.tile_pool(name="pskq", bufs=2, space="PSUM") as PSB,
                    tc.tile_pool(name="psS", bufs=2, space="PSUM") as PSS,
                    tc.tile_pool(name="psO", bufs=1, space="PSUM") as PSO,
                ):
                    wk = WS.tile([128, ND, D], BF16, tag="wreg", name="wk")
                    for kt in range(ND):
                        nc.sync.dma_start(wk[:, kt, :],
                                          wqkv[kt * 128:(kt + 1) * 128, D:2 * D])
                    wq = WS.tile([128, ND, D], BF16, tag="wreg", name="wq")
                    for kt in range(ND):
                        nc.sync.dma_start(wq[:, kt, :],
                                          wqkv[kt * 128:(kt + 1) * 128, 0:D])
                    for ft in range(ND):
                        # K[ft] (both token chunks) and Q[ft]
                        psk = [PSB.tile([128, 512], F32, tag="pskq",
                                        name=f"psk{c}") for c in range(2)]
                        for kt in range(ND):
                            for c in range(2):
                                nc.tensor.matmul(
                                    psk[c][:],
                                    wk[:, kt, ft * 128:(ft + 1) * 128],
                                    xnT[:, kt, c * 512:(c + 1) * 512],
                                    start=(kt == 0), stop=(kt == ND - 1))
                        for c in range(2):
                            nc.vector.tensor_scalar_add(
                                kT[:, ft, c * 512:(c + 1) * 512], psk[c][:],
                                bqk_sb[:, 8 + ft:9 + ft])
                        psq = PSB.tile([128, OWN], F32, tag="pskq", name="psq")
                        for kt in range(ND):
                            nc.tensor.matmul(psq[:],
                                             wq[:, kt, ft * 128:(ft + 1) * 128],
                                             xnT[:, kt, 0:OWN],
                                             start=(kt == 0), stop=(kt == ND - 1))
                        nc.vector.tensor_scalar_add(qT[:, ft, :], psq[:],
                                                    bqk_sb[:, ft:ft + 1])
                        # attention for the two heads living in this ft tile
                        for hh in range(2):
                            h = 2 * ft + hh
                            off = hh * DH
                            kh = kT[off:off + DH, ft, :]
                            qh = qT[off:off + DH, ft, :]
                            expS = AT.tile([128, NT, OWN], BF16, tag="expS",
                                           name="expS", bufs=3)
                            for kp in range(NT // 2):
                                ps = PSS.tile([128, 2, OWN], F32, tag="psS",
                                              name="psS")
                                for j in range(2):
                                    kt = 2 * kp + j
                                    nc.tensor.matmul(
                                        ps[:, j, :],
                                        kh[:, kt * 128:(kt + 1) * 128],
                                        qh[:, :], start=True, stop=True)
                                nc.scalar.activation(
                                    expS[:, 2 * kp:2 * kp + 2, :].rearrange(
                                        "p a b -> p (a b)"),
                                    ps[:, :, :].rearrange("p a b -> p (a b)"),
                                    AF.Exp)
                            # oT_h[d, q] (+ row DH = softmax denominator)
                            po = PSO.tile([DH + 1, OWN], F32, tag="psO",
                                          name="psO")
                            for kt in range(NT):
                                nc.tensor.matmul(
                                    po[:],
                                    vA[:, kt, h * (DH + 1):(h + 1) * (DH + 1)],
                                    expS[:, kt, :],
                                    start=(kt == 0), stop=(kt == NT - 1))
                            rec = SM.tile([1, OWN], F32, tag="rec", name="rec")
                            nc.vector.reciprocal(rec[:], po[DH:DH + 1, :])
                            recb = SM.tile([1, OWN], BF16, tag="recb",
                                           name="recb")
                            nc.vector.tensor_copy(recb[:], rec[:])
                            pb = PSO.tile([DH, OWN], F32, tag="psBC",
                                          name="psBC", bufs=1)
                            nc.tensor.matmul(pb[:], ones64[:], recb[:],
                                             start=True, stop=True)
                            nb = SM.tile([DH, OWN], F32, tag="nb", name="nb")
                            nc.vector.tensor_copy(nb[:], pb[:])
                            nc.vector.tensor_mul(oT[off:off + DH, ft, :],
                                                 po[0:DH, :], nb[:])

            # ---------------- Phase E: O-proj + residual ----------------
            if "E" in _PHASES:
                with (
                    nc.named_scope("phE_oproj"),
                    tc.tile_pool(name="phE", bufs=1) as PE_,
                    tc.tile_pool(name="psE", bufs=4, space="PSUM") as PSE,
                ):
                    wo_sb = PE_.tile([128, ND, D], BF16)
                    for kt in range(ND):
                        nc.sync.dma_start(wo_sb[:, kt, :],
                                          wo[kt * 128:(kt + 1) * 128, :])
                    xres_sb = PE_.tile([128, NQ, D], F32)
                    for qt in range(NQ):
                        nc.gpsimd.dma_start(xres_sb[:, qt, :],
                                            xres[qt * 128:(qt + 1) * 128, :])
                    for qt in range(NQ):
                        for c in range(2):
                            ps = PSE.tile([128, 512], F32, tag="psE", name="psE")
                            for kt in range(ND):
                                nc.tensor.matmul(
                                    ps[:], oT[:, kt, qt * 128:(qt + 1) * 128],
                                    wo_sb[:, kt, c * 512:(c + 1) * 512],
                                    start=(kt == 0), stop=(kt == ND - 1))
                            nc.vector.tensor_add(
                                x2[:, qt, c * 512:(c + 1) * 512], ps[:],
                                xres_sb[:, qt, c * 512:(c + 1) * 512])

            nc.gpsimd.dma_start(b2_sb[:], b2[:])
            nc.gpsimd.dma_start(b1_sb[:], b1[:])
            nc.gpsimd.dma_start(b1s_sb[:], b1s[:])

            # ---------------- Phase F: LN2 + transpose + x2b ----------------
            if "F" in _PHASES:
                with nc.named_scope("phF_ln2"), \
                        tc.tile_pool(name="phF", bufs=4) as PF:
                    for qt in range(NQ):
                        ln_tile(PF, x2[:, qt, :], xn2T, qt)
                        nc.vector.tensor_add(x2b[:, qt, :], x2[:, qt, :],
                                             b2_sb[:])

            # ---------------- Phase G: MLP1 + erf-gelu ----------------
            if "G" in _PHASES:
                with tc.tile_pool(name="mlpG", bufs=1) as MG:
                    g_sb = MG.tile([128, NM, OWN], BF16)
                    with (
                        nc.named_scope("phG_mlp1"),
                        tc.tile_pool(name="w1s", bufs=3) as W1S,
                        tc.tile_pool(name="psG", bufs=8, space="PSUM") as PSG,
                    ):
                        for gp in range(NM // 8):
                            w1ts = []
                            for kt in range(ND):
                                w1t = W1S.tile([128, 1024], BF16, tag="w1t",
                                               name=f"w1t{kt}", bufs=10)
                                nc.sync.dma_start(
                                    w1t[:], w1[kt * 128:(kt + 1) * 128,
                                               gp * 1024:(gp + 1) * 1024])
                                w1ts.append(w1t)
                            for sub in range(2):
                                gm = 2 * gp + sub
                                pss = [PSG.tile([128, OWN], F32, tag="psG",
                                                name=f"psG{mi}")
                                       for mi in range(4)]
                                for kt in range(ND):
                                    for mi in range(4):
                                        nc.tensor.matmul(
                                            pss[mi][:],
                                            w1ts[kt][:, sub * 512 + mi * 128:
                                                     sub * 512 + (mi + 1) * 128],
                                            xn2T[:, kt, :],
                                            start=(kt == 0), stop=(kt == ND - 1))
                                for mi in range(4):
                                    mt = gm * 4 + mi
                                    # exact gelu via erf: g = z*(1+erf(z/sqrt2));
                                    # the 0.5 is folded into W2 on the host.
                                    et = W1S.tile([128, OWN], BF16, tag="et",
                                                  name="et", bufs=4)
                                    nc.scalar.activation(
                                        et[:], pss[mi][:], AF.Erf,
                                        bias=b1s_sb[:, mt:mt + 1],
                                        scale=0.7071067811865476)
                                    zt = W1S.tile([128, OWN], BF16, tag="zt",
                                                  name="zt", bufs=4)
                                    nc.vector.tensor_scalar_add(
                                        zt[:], pss[mi][:], b1_sb[:, mt:mt + 1])
                                    nc.vector.scalar_tensor_tensor(
                                        g_sb[:, mt, :], et[:], 1.0, zt[:],
                                        op0=ALU.add, op1=ALU.mult)

                    # ---------------- Phase H: MLP2 + final residual --------
                    if "H" in _PHASES:
                        with (
                            nc.named_scope("phH_mlp2"),
                            tc.tile_pool(name="w2s", bufs=3) as W2S,
                            tc.tile_pool(name="psH", bufs=8, space="PSUM") as PSH,
                            tc.tile_pool(name="phH", bufs=2) as PH,
                        ):
                            psh = [PSH.tile([128, 512], F32, tag="psH",
                                            name=f"psH{j}") for j in range(8)]
                            for mt in range(NM):
                                w2t = W2S.tile([128, D], BF16, tag="w2t",
                                               name="w2t")
                                nc.sync.dma_start(
                                    w2t[:], w2[mt * 128:(mt + 1) * 128, :])
                                for qt in range(NQ):
                                    for c in range(2):
                                        nc.tensor.matmul(
                                            psh[qt * 2 + c][:],
                                            g_sb[:, mt, qt * 128:(qt + 1) * 128],
                                            w2t[:, c * 512:(c + 1) * 512],
                                            start=(mt == 0), stop=(mt == NM - 1))
                            for qt in range(NQ):
                                ob = PH.tile([128, D], F32, tag="ob", name="ob")
                                for c in range(2):
                                    nc.vector.tensor_add(
                                        ob[:, c * 512:(c + 1) * 512],
                                        psh[qt * 2 + c][:],
                                        x2b[:, qt, c * 512:(c + 1) * 512])
                                nc.sync.dma_start(out[qt * 128:(qt + 1) * 128, :],
                                                  ob[:])

            if "H" not in _PHASES:
                with tc.tile_pool(name="fb", bufs=1) as FB:
                    fbt = FB.tile([128, D], F32)
                    for qt in range(NQ):
                        nc.sync.dma_start(fbt[:], x[qt * 128:(qt + 1) * 128, :])
                        nc.sync.dma_start(out[qt * 128:(qt + 1) * 128, :], fbt[:])

    nc.compile()
    return nc


def _prep_shared(inputs):
    f = lambda k: np.asarray(inputs[k], dtype=np.float32)
    W_qkv, b_qkv = f("W_qkv"), f("b_qkv")
    ln1_g, ln1_b = f("ln1_g"), f("ln1_b")
    ln2_g, ln2_b = f("ln2_g"), f("ln2_b")
    W1, b1 = f("W1"), f("b1")
    W2, b2 = f("W2"), f("b2")
    W_o, b_o = f("W_o"), f("b_o")

    Wq = ln1_g[:, None] * W_qkv
    bq = b_qkv + ln1_b @ W_qkv
    Wq[:, :D] *= SCALE
    bq = bq.copy()
    bq[:D] *= SCALE

    W1e = ln2_g[:, None] * W1
    b1e = b1 + ln2_b @ W1

    bf = ml_dtypes.bfloat16
    return {
        "wqkv": np.ascontiguousarray(Wq, dtype=bf),
        "bqk": np.ascontiguousarray(bq[:2 * D].reshape(16, 128).T,
                                    dtype=np.float32),
        "bv": np.ascontiguousarray(
            np.broadcast_to(bq[2 * D:], (128, D)), dtype=np.float32),
        "wo": np.ascontiguousarray(W_o, dtype=bf),
        "w1": np.ascontiguousarray(W1e, dtype=bf),
        "b1": np.ascontiguousarray(b1e.reshape(NM, 128).T, dtype=np.float32),
        "b1s": np.ascontiguousarray(
            (b1e / np.sqrt(2.0)).reshape(NM, 128).T, dtype=np.float32),
        "w2": np.ascontiguousarray(0.5 * W2, dtype=bf),
        "b2": np.ascontiguousarray(np.broadcast_to(b2, (128, D)),
                                   dtype=np.float32),
    }, b_o


def make_in_maps(inputs):
    shared, b_o = _prep_shared(inputs)
    x = np.asarray(inputs["x"], dtype=np.float32)
    in_maps = []
    for i in range(N_CORES):
        b, h = i // 2, i % 2
        own = x[b, h * OWN:(h + 1) * OWN]
        oth = x[b, (1 - h) * OWN:(2 - h) * OWN]
        m = dict(shared)
        m["x"] = np.ascontiguousarray(
            np.concatenate([own, oth], axis=0), dtype=ml_dtypes.bfloat16)
        m["xres"] = np.ascontiguousarray(own + b_o)
        in_maps.append(m)
    return in_maps


def kernel(**inputs):
    if "nc" not in _CACHE:
        _CACHE["nc"] = _build()
    nc = _CACHE["nc"]
    in_maps = make_in_maps(inputs)
    res = run_bass_kernel_spmd(nc, in_maps, core_ids=list(range(N_CORES)))
    out = np.empty((B, P, D), dtype=np.float32)
    for i in range(N_CORES):
        b, h = i // 2, i % 2
        out[b, h * OWN:(h + 1) * OWN] = res.results[i]["out"]
    return out


# revision 57
# speedup vs baseline: 1.0409x; 1.0034x over previous
"""Trainium2 Bass kernel for a dense transformer block (attention + MLP).

Reference shapes: x [4, 1024, 1024], H=16 heads, DH=64, MLP=4096.

Distribution (8 cores, no collectives): core i handles batch b = i//2,
sequence half h = i%2 (512 query tokens).  Each core receives the full
1024-token context of its batch element with its own 512 tokens permuted
first, recomputes K/V for the whole context (+14% FLOPs, zero comm), and
produces its 512 output rows.  Softmax is order-invariant over keys, so the
permuted context gives exact results.

Per-core pipeline (all matmuls bf16 with f32 PSUM accumulation):
  LN1 (bn_stats) -> DMA-transpose -> V,K,Q projections -> per head:
  S^T = K_h^T.Q_h, exp (scores bounded, no max-subtraction), oT_h = V'_h.A
  (V augmented with a ones column so the softmax denominator is row DH of
  the same accumulation), normalize via a K=1 fp32r broadcast matmul ->
  O-proj + residual -> LN2 -> MLP1 + exact erf-gelu -> MLP2 + residual.

Host-side folds: LN affine into the following weight matrices, attention
scale into W_q/b_q, b_o into the residual input, 0.5 of gelu into W2.
"""

import os
import numpy as np
import ml_dtypes

import concourse.bacc as bacc
import concourse.mybir as mybir
import concourse.tile as tile
from concourse.bass_utils import run_bass_kernel_spmd

B, P, D = 4, 1024, 1024
H, DH = 16, 64
MLP = 4096
SCALE = DH ** -0.5
N_CORES = 8
OWN = P // 2          # query tokens per core
NT = P // 128         # token tiles in context (8)
NQ = OWN // 128       # query token tiles (4)
ND = D // 128         # feature tiles (8)
NM = MLP // 128       # mlp tiles (32)

F32 = mybir.dt.float32
F32R = mybir.dt.float32r
BF16 = mybir.dt.bfloat16
AF = mybir.ActivationFunctionType
ALU = mybir.AluOpType
AX = mybir.AxisListType

_CACHE = {}
_PHASES = os.environ.get("KBENCH_PHASES", "ABEFGH")


def _build():
    nc = bacc.Bacc(None, target_bir_lowering=False, debug=False)

    x = nc.declare_dram_parameter("x", [P, D], BF16, isOutput=False)
    xres = nc.declare_dram_parameter("xres", [OWN, D], F32, isOutput=False)
    wqkv = nc.declare_dram_parameter("wqkv", [D, 3 * D], BF16, isOutput=False)
    bqk = nc.declare_dram_parameter("bqk", [128, 16], F32, isOutput=False)
    bv = nc.declare_dram_parameter("bv", [128, D], F32, isOutput=False)
    wo = nc.declare_dram_parameter("wo", [D, D], BF16, isOutput=False)
    w1 = nc.declare_dram_parameter("w1", [D, MLP], BF16, isOutput=False)
    b1 = nc.declare_dram_parameter("b1", [128, NM], F32, isOutput=False)
    b1s = nc.declare_dram_parameter("b1s", [128, NM], F32, isOutput=False)
    w2 = nc.declare_dram_parameter("w2", [MLP, D], BF16, isOutput=False)
    b2 = nc.declare_dram_parameter("b2", [128, D], F32, isOutput=False)
    out = nc.declare_dram_parameter("out", [OWN, D], F32, isOutput=True)

    with tile.TileContext(nc) as tc:
        with tc.tile_pool(name="persist", bufs=1) as PP:
            # feature-major slabs: slab[p, d, t] = mat[t, d*128+p]
            xnT = PP.tile([128, ND, P], BF16)       # LN1(x) transposed
            qT = PP.tile([128, ND, OWN], BF16)      # queries (pre-scaled)
            kT = PP.tile([128, ND, P], BF16)        # keys
            vA = PP.tile([128, NT, H * (DH + 1)], BF16)  # values + ones col/head
            oT = PP.tile([128, ND, OWN], BF16)      # attn out, feature-major
            x2 = PP.tile([128, NQ, D], F32)         # attn residual state
            x2b = PP.tile([128, NQ, D], F32)        # x2 + b2 (final residual)
            xn2T = PP.tile([128, ND, OWN], BF16)    # LN2(x2) transposed
            bqk_sb = PP.tile([128, 16], F32)
            bv_sb = PP.tile([128, D], F32)
            b1_sb = PP.tile([128, NM], F32)
            b1s_sb = PP.tile([128, NM], F32)
            b2_sb = PP.tile([128, D], F32)
            eps = PP.tile([128, 1], F32)
            nc.vector.memset(eps[:], 1e-5)
            warm = PP.tile([128, 1], F32)
            nc.scalar.sqrt(warm[:], eps[:])
            ones64 = PP.tile([1, DH], BF16)
            nc.vector.memset(ones64[:], 1.0)


            def ln_tile(pool, src, dst_slab, col):
                """LayerNorm src [128, D] (f32) -> bf16, DMA-transposed into
                dst_slab[:, :, col*128:(col+1)*128]."""
                st6 = pool.tile([128, 2, 6], F32, tag="st6", name="st6")
                for c in range(2):
                    nc.vector.bn_stats(st6[:, c, :], src[:, c * 512:(c + 1) * 512])
                mv = pool.tile([128, 2], F32, tag="mv", name="mv")
                nc.vector.bn_aggr(mv[:], st6[:, :, :].rearrange("p a b -> p (a b)"))
                std = pool.tile([128, 1], F32, tag="std", name="std")
                nc.scalar.activation(std[:], mv[:, 1:2], AF.Sqrt,
                                     bias=eps[:, 0:1], scale=1.0)
                rstd = pool.tile([128, 1], F32, tag="rstd", name="rstd")
                nc.vector.reciprocal(rstd[:], std[:])
                nmr = pool.tile([128, 1], F32, tag="nmr", name="nmr")
                nc.vector.scalar_tensor_tensor(nmr[:], mv[:, 0:1], -1.0, rstd[:],
                                               op0=ALU.mult, op1=ALU.mult)
                xnb = pool.tile([128, D], BF16, tag="xnb", name="xnb")
                nc.scalar.activation(xnb[:], src, AF.Identity,
                                     bias=nmr[:, 0:1], scale=rstd[:, 0:1])
                nc.sync.dma_start(dst_slab[:, :, col * 128:(col + 1) * 128],
                                  xnb[:], transpose=True)

            # ------- Phase A: LN1 + transpose + V projection (fused) -------
            if "A" in _PHASES:
                with (
                    nc.named_scope("phA_ln1_v"),
                    tc.tile_pool(name="phA", bufs=3) as PA,
                    tc.tile_pool(name="wstA", bufs=1) as WSA,
                    tc.tile_pool(name="psA", bufs=4, space="PSUM") as PSA,
                ):
                    xts = []
                    for tt in range(NT):
                        xt = PA.tile([128, D], BF16, tag="xt", name=f"xt{tt}",
                                     bufs=NT)
                        if tt < 2:
                            nc.gpsimd.dma_start(xt[:],
                                                x[tt * 128:(tt + 1) * 128, :])
                        xts.append(xt)
                    nc.gpsimd.dma_start(bv_sb[:], bv[:])
                    wv = WSA.tile([128, ND, D], BF16)
                    for kt in range(ND):
                        nc.sync.dma_start(wv[:, kt, :],
                                          wqkv[kt * 128:(kt + 1) * 128,
                                               2 * D:3 * D])
                    for tt in range(2, NT):
                        nc.gpsimd.dma_start(xts[tt][:],
                                            x[tt * 128:(tt + 1) * 128, :])
                    nc.gpsimd.dma_start(bqk_sb[:], bqk[:])
                    vA4 = vA.rearrange("p t (h e) -> p t h e", e=DH + 1)
                    nc.vector.memset(vA4[:, :, :, DH:DH + 1], 1.0)
                    for tt in range(NT):
                        ln_tile(PA, xts[tt][:], xnT, tt)
                        pss = [PSA.tile([128, 512], F32, tag="psA",
                                        name=f"psv{c}") for c in range(2)]
                        for kt in range(ND):
                            for c in range(2):
                                nc.tensor.matmul(
                                    pss[c][:],
                                    xnT[:, kt, tt * 128:(tt + 1) * 128],
                                    wv[:, kt, c * 512:(c + 1) * 512],
                                    start=(kt == 0), stop=(kt == ND - 1))
                        for c in range(2):
                            nc.vector.tensor_add(
                                vA4[:, tt, c * 8:(c + 1) * 8, 0:DH],
                                pss[c][:].rearrange("p (h e) -> p h e", e=DH),
                                bv_sb[:, c * 512:(c + 1) * 512].rearrange(
                                    "p (h e) -> p h e", e=DH))

            # ------- Phase B: K/Q projection + attention (fused per ft) -------
            if "B" in _PHASES:
                with (
                    nc.named_scope("phB_kq_attn"),
                    tc.tile_pool(name="wstage", bufs=2) as WS,
                    tc.tile_pool(name="attn", bufs=2) as AT,
                    tc.tile_pool(name="small", bufs=4) as SM,
                    tc.tile_pool(name="pskq", bufs=2, space="PSUM") as PSB,
                    tc.tile_pool(name="psS", bufs=2, space="PSUM") as PSS,
                    tc.tile_pool(name="psO", bufs=1, space="PSUM") as PSO,
                ):
                    wk = WS.tile([128, ND, D], BF16, tag="wreg", name="wk")
                    for kt in range(ND):
                        nc.sync.dma_start(wk[:, kt, :],
                                          wqkv[kt * 128:(kt + 1) * 128, D:2 * D])
                    wq = WS.tile([128, ND, D], BF16, tag="wreg", name="wq")
                    for kt in range(ND):
                        nc.sync.dma_start(wq[:, kt, :],
                                          wqkv[kt * 128:(kt + 1) * 128, 0:D])
                    for ft in range(ND):
                        # K[ft] (both token chunks) and Q[ft]
                        psk = [PSB.tile([128, 512], F32, tag="pskq",
                                        name=f"psk{c}") for c in range(2)]
                        for kt in range(ND):
                            for c in range(2):
                                nc.tensor.matmul(
                                    psk[c][:],
                                    wk[:, kt, ft * 128:(ft + 1) * 128],
                                    xnT[:, kt, c * 512:(c + 1) * 512],
                                    start=(kt == 0), stop=(kt == ND - 1))
                        for c in range(2):
                            nc.vector.tensor_scalar_add(
                                kT[:, ft, c * 512:(c + 1) * 512], psk[c][:],
                                bqk_sb[:, 8 + ft:9 + ft])
                        psq = PSB.tile([128, OWN], F32, tag="pskq", name="psq")
                        for kt in range(ND):
                            nc.tensor.matmul(psq[:],
                                             wq[:, kt, ft * 128:(ft + 1) * 128],
                                             xnT[:, kt, 0:OWN],
                                             start=(kt == 0), stop=(kt == ND - 1))
                        nc.vector.tensor_scalar_add(qT[:, ft, :], psq[:],
                                                    bqk_sb[:, ft:ft + 1])
                        # attention for the two heads living in this ft tile
                        for hh in range(2):
                            h = 2 * ft + hh
                            off = hh * DH
                            kh = kT[off:off + DH, ft, :]
                            qh = qT[off:off + DH, ft, :]
                            expS = AT.tile([128, NT, OWN], BF16, tag="expS",
                                           name="expS", bufs=3)
                            for kp in range(NT // 2):
                                ps = PSS.tile([128, 2, OWN], F32, tag="psS",
                                              name="psS")
                                for j in range(2):
                                    kt = 2 * kp + j
                                    nc.tensor.matmul(
                                        ps[:, j, :],
                                        kh[:, kt * 128:(kt + 1) * 128],
                                        qh[:, :], start=True, stop=True)
                                nc.scalar.activation(
                                    expS[:, 2 * kp:2 * kp + 2, :].rearrange(
                                        "p a b -> p (a b)"),
                                    ps[:, :, :].rearrange("p a b -> p (a b)"),
                                    AF.Exp)
                            # oT_h[d, q] (+ row DH = softmax denominator)
                            po = PSO.tile([DH + 1, OWN], F32, tag="psO",
                                          name="psO")
                            for kt in range(NT):
                                nc.tensor.matmul(
                                    po[:],
                                    vA[:, kt, h * (DH + 1):(h + 1) * (DH + 1)],
                                    expS[:, kt, :],
                                    start=(kt == 0), stop=(kt == NT - 1))
                            rec = SM.tile([1, OWN], F32, tag="rec", name="rec")
                            nc.vector.reciprocal(rec[:], po[DH:DH + 1, :])
                            recb = SM.tile([1, OWN], BF16, tag="recb",
                                           name="recb")
                            nc.vector.tensor_copy(recb[:], rec[:])
                            pb = PSO.tile([DH, OWN], F32, tag="psBC",
                                          name="psBC", bufs=1)
                            nc.tensor.matmul(pb[:], ones64[:], recb[:],
                                             start=True, stop=True)
                            nb = SM.tile([DH, OWN], F32, tag="nb", name="nb")
                            nc.vector.tensor_copy(nb[:], pb[:])
                            nc.vector.tensor_mul(oT[off:off + DH, ft, :],
                                                 po[0:DH, :], nb[:])

            # ---------------- Phase E: O-proj + residual ----------------
            if "E" in _PHASES:
                with (
                    nc.named_scope("phE_oproj"),
                    tc.tile_pool(name="phE", bufs=1) as PE_,
                    tc.tile_pool(name="psE", bufs=4, space="PSUM") as PSE,
                ):
                    wo_sb = PE_.tile([128, ND, D], BF16)
                    for kt in range(ND):
                        nc.sync.dma_start(wo_sb[:, kt, :],
                                          wo[kt * 128:(kt + 1) * 128, :])
                    xres_sb = PE_.tile([128, NQ, D], F32)
                    for qt in range(NQ):
                        nc.gpsimd.dma_start(xres_sb[:, qt, :],
                                            xres[qt * 128:(qt + 1) * 128, :])
                    for qt in range(NQ):
                        for c in range(2):
                            ps = PSE.tile([128, 512], F32, tag="psE", name="psE")
                            for kt in range(ND):
                                nc.tensor.matmul(
                                    ps[:], oT[:, kt, qt * 128:(qt + 1) * 128],
                                    wo_sb[:, kt, c * 512:(c + 1) * 512],
                                    start=(kt == 0), stop=(kt == ND - 1))
                            nc.vector.tensor_add(
                                x2[:, qt, c * 512:(c + 1) * 512], ps[:],
                                xres_sb[:, qt, c * 512:(c + 1) * 512])

            nc.gpsimd.dma_start(b2_sb[:], b2[:])
            nc.gpsimd.dma_start(b1_sb[:], b1[:])
            nc.gpsimd.dma_start(b1s_sb[:], b1s[:])

            # ---------------- Phase F: LN2 + transpose + x2b ----------------
            if "F" in _PHASES:
                with nc.named_scope("phF_ln2"), \
                        tc.tile_pool(name="phF", bufs=4) as PF:
                    for qt in range(NQ):
                        ln_tile(PF, x2[:, qt, :], xn2T, qt)
                        nc.vector.tensor_add(x2b[:, qt, :], x2[:, qt, :],
                                             b2_sb[:])

            # ---------------- Phase G: MLP1 + erf-gelu ----------------
            if "G" in _PHASES:
                with tc.tile_pool(name="mlpG", bufs=1) as MG:
                    g_sb = MG.tile([128, NM, OWN], BF16)
                    with (
                        nc.named_scope("phG_mlp1"),
                        tc.tile_pool(name="w1s", bufs=3) as W1S,
                        tc.tile_pool(name="psG", bufs=8, space="PSUM") as PSG,
                    ):
                        for gp in range(NM // 8):
                            w1ts = []
                            for kt in range(ND):
                                w1t = W1S.tile([128, 1024], BF16, tag="w1t",
                                               name=f"w1t{kt}", bufs=10)
                                nc.sync.dma_start(
                                    w1t[:], w1[kt * 128:(kt + 1) * 128,
                                               gp * 1024:(gp + 1) * 1024])
                                w1ts.append(w1t)
                            for sub in range(2):
                                gm = 2 * gp + sub
                                pss = [PSG.tile([128, OWN], F32, tag="psG",
                                                name=f"psG{mi}")
                                       for mi in range(4)]
                                for kt in range(ND):
                                    for mi in range(4):
                                        nc.tensor.matmul(
                                            pss[mi][:],
                                            w1ts[kt][:, sub * 512 + mi * 128:
                                                     sub * 512 + (mi + 1) * 128],
                                            xn2T[:, kt, :],
                                            start=(kt == 0), stop=(kt == ND - 1))
                                for mi in range(4):
                                    mt = gm * 4 + mi
                                    # exact gelu via erf: g = z*(1+erf(z/sqrt2));
                                    # the 0.5 is folded into W2 on the host.
                                    et = W1S.tile([128, OWN], BF16, tag="et",
                                                  name="et", bufs=4)
                                    nc.scalar.activation(
                                        et[:], pss[mi][:], AF.Erf,
                                        bias=b1s_sb[:, mt:mt + 1],
                                        scale=0.7071067811865476)
                                    zt = W1S.tile([128, OWN], BF16, tag="zt",
                                                  name="zt", bufs=4)
                                    nc.vector.tensor_scalar_add(
                                        zt[:], pss[mi][:], b1_sb[:, mt:mt + 1])
                                    nc.vector.scalar_tensor_tensor(
                                        g_sb[:, mt, :], et[:], 1.0, zt[:],
                                        op0=ALU.add, op1=ALU.mult)

                    # ---------------- Phase H: MLP2 + final residual --------
                    if "H" in _PHASES:
                        with (
                            nc.named_scope("phH_mlp2"),
                            tc.tile_pool(name="w2s", bufs=3) as W2S,
                            tc.tile_pool(name="psH", bufs=8, space="PSUM") as PSH,
                            tc.tile_pool(name="phH", bufs=2) as PH,
                        ):
                            psh = [PSH.tile([128, 512], F32, tag="psH",
                                            name=f"psH{j}") for j in range(8)]
                            for mt in range(NM):
                                w2t = W2S.tile([128, D], BF16, tag="w2t",
                                               name="w2t")
                                nc.sync.dma_start(
                                    w2t[:], w2[mt * 128:(mt + 1) * 128, :])
                                for qt in range(NQ):
                                    for c in range(2):
                                        nc.tensor.matmul(
                                            psh[qt * 2 + c][:],
                                            g_sb[:, mt, qt * 128:(qt + 1) * 128],
                                            w2t[:, c * 512:(c + 1) * 512],
                                            start=(mt == 0), stop=(mt == NM - 1))
                            for qt in range(NQ):
                                ob = PH.tile([128, D], F32, tag="ob", name="ob")
                                for c in range(2):
                                    nc.vector.tensor_add(
                                        ob[:, c * 512:(c + 1) * 512],
                                        psh[qt * 2 + c][:],
                                        x2b[:, qt, c * 512:(c + 1) * 512])
                                    nc.sync.dma_start(
                                        out[qt * 128:(qt + 1) * 128,
                                            c * 512:(c + 1) * 512],
                                        ob[:, c * 512:(c + 1) * 512])

            if "H" not in _PHASES:
                with tc.tile_pool(name="fb", bufs=1) as FB:
                    fbt = FB.tile([128, D], F32)
                    for qt in range(NQ):
                        nc.sync.dma_start(fbt[:], x[qt * 128:(qt + 1) * 128, :])
                        nc.sync.dma_start(out[qt * 128:(qt + 1) * 128, :], fbt[:])

    nc.compile()
    return nc


def _prep_shared(inputs):
    f = lambda k: np.asarray(inputs[k], dtype=np.float32)
    W_qkv, b_qkv = f("W_qkv"), f("b_qkv")
    ln1_g, ln1_b = f("ln1_g"), f("ln1_b")
    ln2_g, ln2_b = f("ln2_g"), f("ln2_b")
    W1, b1 = f("W1"), f("b1")
    W2, b2 = f("W2"), f("b2")
    W_o, b_o = f("W_o"), f("b_o")

    Wq = ln1_g[:, None] * W_qkv
    bq = b_qkv + ln1_b @ W_qkv
    Wq[:, :D] *= SCALE
    bq = bq.copy()
    bq[:D] *= SCALE

    W1e = ln2_g[:, None] * W1
    b1e = b1 + ln2_b @ W1

    bf = ml_dtypes.bfloat16
    return {
        "wqkv": np.ascontiguousarray(Wq, dtype=bf),
        "bqk": np.ascontiguousarray(bq[:2 * D].reshape(16, 128).T,
                                    dtype=np.float32),
        "bv": np.ascontiguousarray(
            np.broadcast_to(bq[2 * D:], (128, D)), dtype=np.float32),
        "wo": np.ascontiguousarray(W_o, dtype=bf),
        "w1": np.ascontiguousarray(W1e, dtype=bf),
        "b1": np.ascontiguousarray(b1e.reshape(NM, 128).T, dtype=np.float32),
        "b1s": np.ascontiguousarray(
            (b1e / np.sqrt(2.0)).reshape(NM, 128).T, dtype=np.float32),
        "w2": np.ascontiguousarray(0.5 * W2, dtype=bf),
        "b2": np.ascontiguousarray(np.broadcast_to(b2, (128, D)),
                                   dtype=np.float32),
    }, b_o


def make_in_maps(inputs):
    shared, b_o = _prep_shared(inputs)
    x = np.asarray(inputs["x"], dtype=np.float32)
    in_maps = []
    for i in range(N_CORES):
        b, h = i // 2, i % 2
        own = x[b, h * OWN:(h + 1) * OWN]
        oth = x[b, (1 - h) * OWN:(2 - h) * OWN]
        m = dict(shared)
        m["x"] = np.ascontiguousarray(
            np.concatenate([own, oth], axis=0), dtype=ml_dtypes.bfloat16)
        m["xres"] = np.ascontiguousarray(own + b_o)
        in_maps.append(m)
    return in_maps


def kernel(**inputs):
    if "nc" not in _CACHE:
        _CACHE["nc"] = _build()
    nc = _CACHE["nc"]
    in_maps = make_in_maps(inputs)
    res = run_bass_kernel_spmd(nc, in_maps, core_ids=list(range(N_CORES)))
    out = np.empty((B, P, D), dtype=np.float32)
    for i in range(N_CORES):
        b, h = i // 2, i % 2
        out[b, h * OWN:(h + 1) * OWN] = res.results[i]["out"]
    return out
